# revision 1
# baseline (speedup 1.0000x reference)
"""Causal self-attention TRN2 Bass kernel.

Problem: B=4, T=2048, C=1024, H=16 heads, D=64 (fp32 in/out).

Sharding (8 cores): core i handles batch b = i//2 and heads
8*(i%2) .. 8*(i%2)+8  (8 heads, 512 features). Each core:
  qkv_local = x[b] @ W_attn[:, cols] (+bias)       [2048, 512] x3
  attention over its 8 heads (causal, T=2048)
  partial_out = y_local @ W_proj[rows, :]          [2048, 1024]
Host: out[b] = partial(core 2b) + partial(core 2b+1) + bias_terms.

Matmul operands are bf16 (fp32 accumulation in PSUM): same PE rate as
fp32r at free>=256, but LDWEIGHTS gets fast-weight-load (disabled for
fp32) and the small diagonal blocks avoid the fp32r free<256 penalty.

Device layouts (SBUF partition dim first):
  qT/kT: [128 (head-pair 2x64 d), t]  -- W.T @ x^T matmul outputs
  v:     [128 (t mod 128), tb, head, 66]  (cols 64,65 = 1.0 -> denominator)
  S^T:   [128 k, 2 (head), 512 q] psum pairs; ONE exp per (j,kb) on ACT;
  att@v: Y^T [66, 512] psum per head, row 64 = softmax denominator.
Normalize: copy Y to SBUF (frees the PSUM bank fast), then split DVE
reciprocal + gpsimd partition_broadcast + mul.  Scheduling: projection of
chunk tcx is deferred one chunk and woven, together with QKV emission for
chunk tcx+1, into the attention kb-loop, so the PE never idles long
enough for the HAM clock gate to re-throttle.
"""
import numpy as np
from contextlib import ExitStack

import jax
import concourse.bass as bass
import concourse.tile as tile
from concourse import bacc, mybir
from concourse.bass_utils import run_bass_kernel_spmd

jax.config.update("jax_compilation_cache_dir", "/tmp/jaxcache")
jax.config.update("jax_persistent_cache_min_entry_size_bytes", -1)
jax.config.update("jax_persistent_cache_min_compile_time_secs", 0.0)

B, T, C, H, D = 4, 2048, 1024, 16, 64
NCORES = 8
HPC = 8            # heads per core
FL = HPC * D       # 512 local features per core
NTC = 4            # 512-token chunks per core
NTB = 16           # 128-token blocks per core
F32 = mybir.dt.float32
BF16 = mybir.dt.bfloat16
NPBF16 = mybir.dt.np(BF16)
AF = mybir.ActivationFunctionType

_CACHED_NC = None


def _build(reps=1):
    nc = bacc.Bacc("TRN2", target_bir_lowering=False, debug=False,
                   num_devices=NCORES)

    xt = nc.dram_tensor("xt", [C, T], BF16, kind="ExternalInput").ap()
    wq = nc.dram_tensor("wq", [C, FL], BF16, kind="ExternalInput").ap()
    wk = nc.dram_tensor("wk", [C, FL], BF16, kind="ExternalInput").ap()
    wv = nc.dram_tensor("wv", [C, FL], BF16, kind="ExternalInput").ap()
    wp = nc.dram_tensor("wp", [FL, C], BF16, kind="ExternalInput").ap()
    bq = nc.dram_tensor("bq", [128, 4], F32, kind="ExternalInput").ap()
    bk = nc.dram_tensor("bk", [128, 4], F32, kind="ExternalInput").ap()
    tri = nc.dram_tensor("tri", [128, 128], F32, kind="ExternalInput").ap()
    out = nc.dram_tensor("out", [T, C], F32, kind="ExternalOutput").ap()

    with tile.TileContext(nc) as tc, ExitStack() as ctx:
        ctx.enter_context(nc.allow_low_precision(reason="bf16 matmuls"))
        singles = ctx.enter_context(tc.tile_pool(name="singles", bufs=1))
        xt_pool = ctx.enter_context(tc.tile_pool(name="xt", bufs=9))
        qT_pool = ctx.enter_context(tc.tile_pool(name="qT", bufs=8))
        attT_pool = ctx.enter_context(tc.tile_pool(name="attT", bufs=4))
        yT_pool = ctx.enter_context(tc.tile_pool(name="yT", bufs=3))
        rc_pool = ctx.enter_context(tc.tile_pool(name="rc", bufs=4))
        bcs_pool = ctx.enter_context(tc.tile_pool(name="bcs", bufs=4))
        yc_pool = ctx.enter_context(tc.tile_pool(name="yc", bufs=4))
        o_pool = ctx.enter_context(tc.tile_pool(name="o", bufs=3))
        # qkv and proj accumulators share one double-buffered pool (2 banks)
        ps_acc = ctx.enter_context(tc.tile_pool(name="ps_acc", bufs=2, space="PSUM"))
        ps_s = ctx.enter_context(tc.tile_pool(name="ps_s", bufs=2, space="PSUM"))
        ps_y = ctx.enter_context(tc.tile_pool(name="ps_y", bufs=2, space="PSUM"))

        wq_sb = singles.tile([128, 8, FL], BF16)
        wk_sb = singles.tile([128, 8, FL], BF16)
        wv_sb = singles.tile([128, 8, FL], BF16)
        wp_sb = singles.tile([128, 4, C], BF16)
        wq_r = wq.rearrange("(cc p) f -> p cc f", p=128)
        wk_r = wk.rearrange("(cc p) f -> p cc f", p=128)
        wv_r = wv.rearrange("(cc p) f -> p cc f", p=128)
        wp_r = wp.rearrange("(j p) o -> p j o", p=128)
        for cc in range(8):
            nc.scalar.dma_start(out=wq_sb[:, cc], in_=wq_r[:, cc])
            nc.scalar.dma_start(out=wk_sb[:, cc], in_=wk_r[:, cc])
            nc.scalar.dma_start(out=wv_sb[:, cc], in_=wv_r[:, cc])
        for j in range(4):
            nc.scalar.dma_start(out=wp_sb[:, j], in_=wp_r[:, j])
        bq_sb = singles.tile([128, 4], F32)
        bk_sb = singles.tile([128, 4], F32)
        tri_sb = singles.tile([128, 128], F32)
        nc.scalar.dma_start(out=bq_sb, in_=bq)
        nc.scalar.dma_start(out=bk_sb, in_=bk)
        nc.scalar.dma_start(out=tri_sb, in_=tri)

        # kT: [128 (pair-feature), j (head pair), t]
        kT_sb = singles.tile([128, 4, T], BF16)
        # v: [128 (t%128), tb, head, 66]; cols 64,65 stay 1.0 -> denominator
        v_sb = singles.tile([128, NTB, HPC, 66], BF16)
        nc.vector.memset(v_sb, 1.0)

        for rep in range(reps):
            # No inter-rep barrier: Tile's dependency tracking serializes the
            # kT/v singles rewrites against the previous rep's readers, and
            # the next rep's QKV prologue overlaps this rep's projection tail.
            qts_all = {}
            yts_all = {}

            def qkv_units(tcx, rep=rep, qts_all=qts_all):
                """Yield thunks; each emits one QKV work unit for chunk tcx."""
                t0 = tcx * 512
                qts = qts_all[tcx] = [
                    qT_pool.tile([128, 512], BF16, tag="qT",
                                 name=f"qt{rep}_{tcx}_{j}")
                    for j in range(4)
                ]
                xts = [xt_pool.tile([128, 512], BF16, tag="xt",
                                    name=f"xt{rep}_{tcx}_{cc}")
                       for cc in range(8)]

                def dmas(xts=xts, t0=t0):
                    for cc in range(8):
                        nc.sync.dma_start(
                            out=xts[cc],
                            in_=xt[cc * 128:(cc + 1) * 128, t0:t0 + 512])
                yield dmas

                def qk_group(w_sb, dest, bias, j):
                    p = ps_acc.tile([128, 512], F32, tag="acc", name="pqk")
                    for cc in range(8):
                        nc.tensor.matmul(
                            p, w_sb[:, cc, j * 128:(j + 1) * 128],
                            xts[cc], start=(cc == 0), stop=(cc == 7))
                    nc.vector.tensor_scalar_add(dest, p, bias)

                for j in range(4):
                    def uq(j=j, qk=qk_group):
                        qk(wq_sb, qts[j], bq_sb[:, j:j + 1], j)
                    yield uq

                    def uk(j=j, t0=t0, qk=qk_group):
                        qk(wk_sb, kT_sb[:, j, t0:t0 + 512],
                           bk_sb[:, j:j + 1], j)
                    yield uk

                for tb_rel in range(4):
                    def uv(tb_rel=tb_rel, xts=xts, tcx=tcx):
                        tb = tcx * 4 + tb_rel
                        pv = ps_acc.tile([128, 512], F32, tag="acc", name="pv")
                        for cc in range(8):
                            nc.tensor.matmul(
                                pv, xts[cc][:, tb_rel * 128:(tb_rel + 1) * 128],
                                wv_sb[:, cc, :], start=(cc == 0), stop=(cc == 7))
                        for j in range(4):
                            nc.vector.tensor_copy(
                                v_sb[:, tb, 2 * j:2 * j + 2, 0:64],
                                pv[:, j * 128:(j + 1) * 128].rearrange(
                                    "p (i d) -> p i d", i=2))
                    yield uv

            def proj_units(tcx, rep=rep, yts_all=yts_all):
                """Yield thunks; each emits one projection unit of chunk tcx."""
                yt = yts_all[tcx]
                for tb_rel in range(4):
                    for oc in range(2):
                        def up(tb_rel=tb_rel, oc=oc, yt=yt, tcx=tcx):
                            pp = ps_acc.tile([128, 512], F32, tag="acc", name="pp")
                            for j in range(4):
                                nc.tensor.matmul(
                                    pp, yt[:, j, tb_rel * 128:(tb_rel + 1) * 128],
                                    wp_sb[:, j, oc * 512:(oc + 1) * 512],
                                    start=(j == 0), stop=(j == 3))
                            po = o_pool.tile([128, 512], F32, tag="o",
                                             name=f"po{rep}_{tcx}_{tb_rel}_{oc}")
                            nc.vector.tensor_copy(po, pp)
                            nc.sync.dma_start(
                                out=out[tcx * 512 + tb_rel * 128:
                                        tcx * 512 + (tb_rel + 1) * 128,
                                        oc * 512:(oc + 1) * 512],
                                in_=po)
                        yield up

            # Deferred softmax-normalize work, paced one op per kb slot so
            # the DVE FIFO never gets a multi-us reciprocal burst in front
            # of the PSUM-releasing copies the PE is waiting on.  Entries are
            # (tcx, thunk); the queue is chunk-ordered (FIFO).
            pending_norm = []

            def pump_norm():
                n = 2 if len(pending_norm) > 8 else 1
                for _ in range(n):
                    if pending_norm:
                        pending_norm.pop(0)[1]()

            def drain_norm(upto_tcx):
                # Emit every deferred normalize for chunks <= upto_tcx NOW:
                # a reader emitted before its writer sees no dependency.
                while pending_norm and pending_norm[0][0] <= upto_tcx:
                    pending_norm.pop(0)[1]()

            # Prologue: QKV(0) fully.
            for u in qkv_units(0):
                u()

            for tcx in range(NTC):
                qts = qts_all[tcx]
                yt = yts_all[tcx] = yT_pool.tile([128, 4, 512], BF16, tag="yT",
                                                 name=f"yt{rep}_{tcx}")
                filler = []
                if tcx + 1 < NTC:
                    filler += list(qkv_units(tcx + 1))
                # Projections are deferred into later chunks' attention loops;
                # chunk 3 (no QKV filler, exp-bound) takes two of them.  The
                # drain guard emits yt's deferred normalize muls first (a
                # reader emitted before its writer sees no dependency).
                if tcx == 1:
                    filler += [lambda: drain_norm(0)]
                    filler += list(proj_units(0))
                elif tcx == 3:
                    filler += [lambda: drain_norm(2)]
                    filler += list(proj_units(1))
                    filler += list(proj_units(2))
                nkb = 4 * tcx + 4
                n_slots = 4 * nkb
                emitted = 0
                slot = 0

                for j in range(4):
                    Y = [ps_y.tile([66, 512], F32, tag="y",
                                   name=f"Y{rep}_{tcx}_{j}_{h}")
                         for h in range(2)]
                    for kb in range(nkb):
                        want = ((slot + 1) * len(filler)) // (n_slots + 1)
                        while emitted < want:
                            filler[emitted]()
                            emitted += 1
                        slot += 1

                        jj = kb - 4 * tcx
                        c0 = max(jj, 0) * 128
                        S = ps_s.tile([128, 2, 512], F32, tag="s",
                                      name=f"S{rep}_{tcx}_{j}_{kb}")
                        for h in range(2):
                            nc.tensor.matmul(
                                S[:, h, c0:512],
                                kT_sb[h * 64:(h + 1) * 64, j,
                                      kb * 128:(kb + 1) * 128],
                                qts[j][h * 64:(h + 1) * 64, c0:512],
                                start=True, stop=True,
                                tile_position=(h * 64, 0))
                        if jj >= 0:
                            for h in range(2):
                                nc.vector.tensor_add(
                                    S[:, h, jj * 128:(jj + 1) * 128],
                                    S[:, h, jj * 128:(jj + 1) * 128], tri_sb)
                        att = attT_pool.tile([128, 2, 512], BF16, tag="attT",
                                             name=f"attT{rep}_{tcx}_{j}_{kb}")
                        nc.scalar.activation(
                            att[:, :, c0:512],
                            S[:, :, c0:512], AF.Exp)
                        for h in range(2):
                            nc.tensor.matmul(
                                Y[h][:, c0:512],
                                v_sb[:, kb, 2 * j + h, :],
                                att[:, h, c0:512],
                                start=(kb == 0), stop=(kb == nkb - 1))
                        pump_norm()
                    # Copy both Y tiles to SBUF immediately: the copies are
                    # the only readers gating the PSUM-bank release, so the
                    # next j's att@v never waits on the normalize chain.  The
                    # reciprocal/broadcast/mul are deferred into pending_norm
                    # and paced one per kb slot.
                    ycs = []
                    for h in range(2):
                        yc = yc_pool.tile([66, 512], F32, tag="yc",
                                          name=f"yc{rep}_{tcx}_{j}_{h}")
                        nc.vector.tensor_copy(yc, Y[h])
                        ycs.append(yc)

                    def norm_thunks(j=j, ycs=ycs, yt=yt, rep=rep, tcx=tcx):
                        for h in range(2):
                            yc = ycs[h]
                            rc = rc_pool.tile([1, 512], F32, tag="rc",
                                              name=f"rc{rep}_{tcx}_{j}_{h}")
                            for sg in range(4):
                                def useg(sg=sg, rc=rc, yc=yc):
                                    nc.vector.reciprocal(
                                        rc[:, sg * 128:(sg + 1) * 128],
                                        yc[64:65, sg * 128:(sg + 1) * 128])
                                yield useg

                            def ubm(h=h, rc=rc, yc=yc):
                                bcs = bcs_pool.tile([64, 512], F32, tag="bcs",
                                                    name=f"bcs{rep}_{tcx}_{j}_{h}")
                                nc.gpsimd.partition_broadcast(bcs, rc)
                                nc.vector.tensor_mul(
                                    yt[h * 64:(h + 1) * 64, j, :],
                                    yc[0:64, :], bcs)
                            yield ubm
                    pending_norm.extend((tcx, t) for t in norm_thunks())
                while emitted < len(filler):
                    filler[emitted]()
                    emitted += 1

            # Drain deferred normalizes (tail projection needs yt of the
            # last chunk), then emit the tail projection.
            drain_norm(NTC - 1)
            for u in proj_units(NTC - 1):
                u()

    nc.compile()
    return nc


def _get_nc():
    global _CACHED_NC
    if _CACHED_NC is None:
        _CACHED_NC = _build()
    return _CACHED_NC


def make_in_maps(x, W_attn, b_attn, W_proj):
    x = np.asarray(x, np.float32)
    W_attn = np.asarray(W_attn, np.float32)
    b_attn = np.asarray(b_attn, np.float32)
    scale = np.float32(1.0 / np.sqrt(D))
    tri = np.where(np.arange(128)[None, :] >= np.arange(128)[:, None],
                   np.float32(0.0), np.float32(-1e4)).astype(np.float32)
    xts = [np.ascontiguousarray(x[b].T.astype(NPBF16)) for b in range(B)]
    in_maps = []
    for core in range(NCORES):
        b = core // 2
        hs = (core % 2) * FL
        qc = slice(hs, hs + FL)
        kc = slice(C + hs, C + hs + FL)
        vc = slice(2 * C + hs, 2 * C + hs + FL)
        in_maps.append({
            "xt": xts[b],
            "wq": np.ascontiguousarray((W_attn[:, qc] * scale).astype(NPBF16)),
            "wk": np.ascontiguousarray(W_attn[:, kc].astype(NPBF16)),
            "wv": np.ascontiguousarray(W_attn[:, vc].astype(NPBF16)),
            "wp": np.ascontiguousarray(
                np.asarray(W_proj, np.float32)[hs:hs + FL, :].astype(NPBF16)),
            "bq": np.ascontiguousarray((b_attn[qc] * scale).reshape(4, 128).T),
            "bk": np.ascontiguousarray(b_attn[kc].reshape(4, 128).T),
            "tri": tri,
        })
    return in_maps


def kernel(x, W_attn, b_attn, W_proj, b_proj):
    x = np.asarray(x, np.float32)
    W_attn = np.asarray(W_attn, np.float32)
    b_attn = np.asarray(b_attn, np.float32)
    W_proj = np.asarray(W_proj, np.float32)
    b_proj = np.asarray(b_proj, np.float32)

    nc = _get_nc()
    in_maps = make_in_maps(x, W_attn, b_attn, W_proj)
    res = run_bass_kernel_spmd(nc, in_maps, list(range(NCORES)))
    outs = [res.results[i]["out"] for i in range(NCORES)]
    y = np.stack([outs[2 * b] + outs[2 * b + 1] for b in range(B)])
    # v-bias folds through attention (rows sum to 1) into a constant output
    # bias: b_proj + b_v @ W_proj.
    bias_out = b_proj + b_attn[2 * C:] @ W_proj
    return (y + bias_out[None, None, :]).astype(np.float32)

